# revision 1
# baseline (speedup 1.0000x reference)
"""Trainium2 Bass kernel for nn_AttentionModel (greedy pointer-attention decode).

Contract: kernel(**inputs) takes FULL inputs (B=1024), shards batch across 8
NeuronCores (128 items each, SPMD), runs the 199-step greedy decode on-device,
returns full (1024, 199, 200) float32 log_p.

Per-core dataflow (batch-on-partitions, b=128):
  precompute: emb2 = emb + pref -> DRAM;  kvl = emb2 @ W_node -> DRAM (gK|gV|lK)
              fixed2 = mean(emb2) @ W_fixed + first @ W_step[:256]
  per step  : stream kvl chunks from HBM; DVE does batched dot-products
              (multiply + strided reduce); ACT does exp/tanh/ln; PE does the
              shared-weight matmuls (cur @ W_step[256:], glimpse @ W_out) and
              transposes; argmax via DVE max/max_index; visited mask kept as a
              0/-1e9 addend; cur_emb gathered by indirect DMA with on-device
              computed row offsets.
"""
import numpy as np

import concourse.bass as bass
from concourse import bacc
import concourse.tile as tile
from concourse import mybir
from concourse.bass import IndirectOffsetOnAxis
from concourse.bass_utils import run_bass_kernel_spmd

dt = mybir.dt
F32 = dt.float32
AX = mybir.AxisListType
OP = mybir.AluOpType
ACTF = mybir.ActivationFunctionType

B, N, D, H = 1024, 200, 256, 8
d = D // H                      # 32
NCORES = 8
BS = B // NCORES                # 128 items per core
T = N - 1                       # 199 decode steps
START = 24
NEG = -1e9
CHUNK = 20                      # n-chunk for streaming kvl
NCH = N // CHUNK                # 10 chunks
ISD = 1.0 / np.sqrt(d).item()   # 1/sqrt(32)
ISD32 = float(np.float32(1.0 / np.sqrt(32.0)))
ISD256 = 0.0625                 # 1/sqrt(256), exact


def _build():
    nc = bacc.Bacc("TRN2", target_bir_lowering=False, debug=False)

    emb_in = nc.dram_tensor("embeddings", [BS, N, D], F32, kind="ExternalInput").ap()
    pref_in = nc.dram_tensor("pref_embed", [D], F32, kind="ExternalInput").ap()
    wnode_in = nc.dram_tensor("W_node", [D, 3 * D], F32, kind="ExternalInput").ap()
    wfix_in = nc.dram_tensor("W_fixed", [D, D], F32, kind="ExternalInput").ap()
    wstep_in = nc.dram_tensor("W_step", [2 * D, D], F32, kind="ExternalInput").ap()
    wout_in = nc.dram_tensor("W_out", [D, D], F32, kind="ExternalInput").ap()

    out = nc.dram_tensor("log_p", [BS, T * N], F32, kind="ExternalOutput").ap()

    emb2_d = nc.dram_tensor("emb2_d", [BS * N, D], F32).ap()
    kvl_d = nc.dram_tensor("kvl_d", [BS, N, 3 * D], F32).ap()

    with tile.TileContext(nc) as tc:
        with (
            tc.tile_pool(name="wpool", bufs=1) as wpool,      # persistent weights/state
            tc.tile_pool(name="stream", bufs=4) as stream,    # kvl chunks
            tc.tile_pool(name="prod", bufs=2) as prodp,       # TT products
            tc.tile_pool(name="work", bufs=2) as work,        # small transient tiles
            tc.tile_pool(name="psum", bufs=2, space="PSUM") as psp,
            tc.tile_pool(name="psum1", bufs=2, space="PSUM") as psp1,
        ):
            # ---------------- persistent tiles ----------------
            wn_sb = wpool.tile([128, 2, 3 * D], F32)    # W_node as [c-tile, 2, 768]
            nc.sync.dma_start(wn_sb[:, 0, :], wnode_in[0:128, :])
            nc.sync.dma_start(wn_sb[:, 1, :], wnode_in[128:256, :])
            w2_sb = wpool.tile([128, 2, D], F32)        # W_step[256:512] c-tiles
            nc.sync.dma_start(w2_sb[:, 0, :], wstep_in[256:384, :])
            nc.sync.dma_start(w2_sb[:, 1, :], wstep_in[384:512, :])
            wo_sb = wpool.tile([128, 2, D], F32)        # W_out c-tiles
            nc.sync.dma_start(wo_sb[:, 0, :], wout_in[0:128, :])
            nc.sync.dma_start(wo_sb[:, 1, :], wout_in[128:256, :])
            wf_sb = wpool.tile([128, 2, D], F32)        # W_fixed c-tiles
            nc.sync.dma_start(wf_sb[:, 0, :], wfix_in[0:128, :])
            nc.sync.dma_start(wf_sb[:, 1, :], wfix_in[128:256, :])
            ws1_sb = wpool.tile([128, 2, D], F32)       # W_step[0:256] c-tiles
            nc.sync.dma_start(ws1_sb[:, 0, :], wstep_in[0:128, :])
            nc.sync.dma_start(ws1_sb[:, 1, :], wstep_in[128:256, :])

            pref_sb = wpool.tile([128, D], F32)
            nc.sync.dma_start(
                pref_sb[:],
                pref_in.rearrange("(o f) -> o f", o=1).broadcast_to([128, D]),
            )

            ident = wpool.tile([128, 128], F32)         # identity for PE transpose
            io_c = wpool.tile([128, 128], dt.int32)
            nc.gpsimd.iota(io_c[:], pattern=[[1, 128]], channel_multiplier=0)
            io_r = wpool.tile([128, 1], dt.int32)
            nc.gpsimd.iota(io_r[:], pattern=[[0, 1]], channel_multiplier=1)
            id_i = wpool.tile([128, 128], dt.int32)
            nc.vector.tensor_tensor(id_i[:], io_c[:], io_r[:].broadcast_to([128, 128]), op=OP.is_equal)
            nc.vector.tensor_copy(ident[:], id_i[:])

            iota_n = wpool.tile([128, N], dt.int32)     # 0..199 per partition
            nc.gpsimd.iota(iota_n[:], pattern=[[1, N]], channel_multiplier=0)
            iota_row = wpool.tile([128, 1], dt.int32)   # p*N
            nc.gpsimd.iota(iota_row[:], pattern=[[0, 1]], channel_multiplier=N)

            amask = wpool.tile([128, N], F32)           # visited addend 0/-1e9
            nc.vector.memset(amask[:], 0.0)
            nc.vector.memset(amask[:, START:START + 1], NEG)

            fixed2 = wpool.tile([128, D], F32)
            first_sb = wpool.tile([128, D], F32)
            q_sb = wpool.tile([128, D], F32)
            cur_sb = wpool.tile([128, D], F32)

            # ---------------- precompute: emb2 + kvl ----------------
            emb_rows = emb_in.rearrange("b n c -> (b n) c")   # [25600, 256]
            ROWT = BS * N // 128                              # 200 row-tiles

            def pre_body(rt):
                erow = work.tile([128, D], F32, tag="erow")
                nc.sync.dma_start(erow[:], emb_rows[bass.ds(rt * 128, 128), :])
                e2 = work.tile([128, D], F32, tag="e2")
                nc.vector.tensor_tensor(e2[:], erow[:], pref_sb[:], op=OP.add)
                nc.sync.dma_start(emb2_d[bass.ds(rt * 128, 128), :], e2[:])
                # transpose e2 -> e2T (2 c-tiles)
                e2T = work.tile([128, 2, 128], F32, tag="e2T")
                for ci in range(2):
                    tp = psp1.tile([128, 128], F32, tag="tp")
                    nc.tensor.transpose(tp[:], e2[:, ci * 128:(ci + 1) * 128], ident[:])
                    nc.vector.tensor_copy(e2T[:, ci, :], tp[:])
                # kvl row-tile = e2 @ W_node  (f split 2x384)
                kv = work.tile([128, 3 * D], F32, tag="kv")
                for fh in range(2):
                    pm = psp.tile([128, 384], F32, tag="ps")
                    nc.tensor.matmul(pm[:], e2T[:, 0, :], wn_sb[:, 0, fh * 384:(fh + 1) * 384], start=True, stop=False)
                    nc.tensor.matmul(pm[:], e2T[:, 1, :], wn_sb[:, 1, fh * 384:(fh + 1) * 384], start=False, stop=True)
                    nc.vector.tensor_copy(kv[:, fh * 384:(fh + 1) * 384], pm[:])
                nc.sync.dma_start(kvl_d.rearrange("b n c -> (b n) c")[bass.ds(rt * 128, 128), :], kv[:])

            tc.For_i_unrolled(0, ROWT, 1, pre_body, max_unroll=4)

            # ---------------- fixed2 ----------------
            macc = wpool.tile([128, D], F32)
            emb2_bnc = emb2_d.rearrange("(b n) c -> b n c", b=BS)
            for c in range(NCH):
                ech = stream.tile([128, CHUNK, D], F32, tag="stream")
                nc.sync.dma_start(ech[:], emb2_bnc[:, c * CHUNK:(c + 1) * CHUNK, :])
                part = work.tile([128, D], F32, tag="mpart")
                nc.vector.tensor_reduce(part[:], ech[:].transpose([0, 2, 1]), axis=AX.X, op=OP.add)
                if c == 0:
                    nc.vector.tensor_copy(macc[:], part[:])
                else:
                    nc.vector.tensor_tensor(macc[:], macc[:], part[:], op=OP.add)
            nc.vector.tensor_scalar(macc[:], macc[:], 1.0 / N, None, op0=OP.mult)
            nc.sync.dma_start(first_sb[:], emb2_bnc[:, START, :])

            fT = work.tile([128, 2, 128], F32, tag="fT")
            mT = work.tile([128, 2, 128], F32, tag="mT")
            for ci in range(2):
                tp = psp1.tile([128, 128], F32, tag="tp")
                nc.tensor.transpose(tp[:], macc[:, ci * 128:(ci + 1) * 128], ident[:])
                nc.vector.tensor_copy(mT[:, ci, :], tp[:])
                tp2 = psp1.tile([128, 128], F32, tag="tp")
                nc.tensor.transpose(tp2[:], first_sb[:, ci * 128:(ci + 1) * 128], ident[:])
                nc.vector.tensor_copy(fT[:, ci, :], tp2[:])
            pf = psp.tile([128, D], F32, tag="ps")
            nc.tensor.matmul(pf[:], mT[:, 0, :], wf_sb[:, 0, :], start=True, stop=False)
            nc.tensor.matmul(pf[:], mT[:, 1, :], wf_sb[:, 1, :], start=False, stop=False)
            nc.tensor.matmul(pf[:], fT[:, 0, :], ws1_sb[:, 0, :], start=False, stop=False)
            nc.tensor.matmul(pf[:], fT[:, 1, :], ws1_sb[:, 1, :], start=False, stop=True)
            nc.vector.tensor_copy(fixed2[:], pf[:])

            # q(t=0): cur = first_emb
            nc.vector.tensor_copy(cur_sb[:], first_sb[:])

            def q_from_cur():
                cT = work.tile([128, 2, 128], F32, tag="cT")
                for ci in range(2):
                    tp = psp1.tile([128, 128], F32, tag="tp")
                    nc.tensor.transpose(tp[:], cur_sb[:, ci * 128:(ci + 1) * 128], ident[:])
                    nc.scalar.copy(cT[:, ci, :], tp[:])
                pq = psp.tile([128, D], F32, tag="ps")
                nc.tensor.matmul(pq[:], cT[:, 0, :], w2_sb[:, 0, :], start=True, stop=False)
                nc.tensor.matmul(pq[:], cT[:, 1, :], w2_sb[:, 1, :], start=False, stop=True)
                nc.scalar.activation(q_sb[:], pq[:], ACTF.Copy)
                nc.vector.tensor_tensor(q_sb[:], q_sb[:], fixed2[:], op=OP.add)

            q_from_cur()

            # ---------------- decode steps ----------------
            compat = wpool.tile([128, H, N], F32)
            attn = wpool.tile([128, H, N], F32)
            logits = wpool.tile([128, N], F32)
            gl_part = wpool.tile([128, NCH, D], F32)
            glimpse = wpool.tile([128, D], F32)

            def step_body(s):
                qb = q_sb[:].rearrange("p (o f) -> p o f", o=1).broadcast_to([128, CHUNK, D])
                # --- compat: per-head dots with gK ---
                for c in range(NCH):
                    kc = stream.tile([128, CHUNK, D], F32, tag="stream")
                    nc.sync.dma_start(kc[:], kvl_d[:, c * CHUNK:(c + 1) * CHUNK, 0:D])
                    pr = prodp.tile([128, CHUNK, D], F32, tag="prod")
                    nc.gpsimd.tensor_tensor(pr[:], kc[:], qb, op=OP.mult)
                    nc.vector.tensor_reduce(
                        compat[:, :, c * CHUNK:(c + 1) * CHUNK].transpose([0, 2, 1]),
                        pr[:].rearrange("p n (h e) -> p n h e", h=H),
                        axis=AX.X, op=OP.add)
                # scale + mask + softmax over n (per head)
                ab = amask[:].rearrange("p (o n) -> p o n", o=1).broadcast_to([128, H, N])
                nc.vector.tensor_scalar(compat[:], compat[:], ISD32, None, op0=OP.mult)
                nc.vector.tensor_tensor(compat[:], compat[:], ab, op=OP.add)
                mh = work.tile([128, H], F32, tag="mh")
                nc.vector.tensor_reduce(mh[:], compat[:], axis=AX.X, op=OP.max)
                nc.vector.tensor_tensor(
                    compat[:], compat[:],
                    mh[:].rearrange("p (h o) -> p h o", o=1).broadcast_to([128, H, N]),
                    op=OP.subtract)
                nc.scalar.activation(attn[:], compat[:], ACTF.Exp)
                sh = work.tile([128, H], F32, tag="sh")
                nc.vector.tensor_reduce(sh[:], attn[:], axis=AX.X, op=OP.add)
                rh = work.tile([128, H], F32, tag="rh")
                nc.vector.reciprocal(rh[:], sh[:])
                nc.vector.tensor_tensor(
                    attn[:], attn[:],
                    rh[:].rearrange("p (h o) -> p h o", o=1).broadcast_to([128, H, N]),
                    op=OP.mult)
                # --- glimpse: attn-weighted gV ---
                for c in range(NCH):
                    vc = stream.tile([128, CHUNK, D], F32, tag="stream")
                    nc.sync.dma_start(vc[:], kvl_d[:, c * CHUNK:(c + 1) * CHUNK, D:2 * D])
                    av = attn[:, :, c * CHUNK:(c + 1) * CHUNK].transpose([0, 2, 1]) \
                        .rearrange("p n (h o) -> p n h o", o=1).broadcast_to([128, CHUNK, H, d])
                    pr = prodp.tile([128, CHUNK, D], F32, tag="prod")
                    nc.gpsimd.tensor_tensor(pr[:].rearrange("p n (h e) -> p n h e", h=H), vc[:].rearrange("p n (h e) -> p n h e", h=H), av, op=OP.mult)
                    nc.vector.tensor_reduce(gl_part[:, c, :], pr[:].transpose([0, 2, 1]), axis=AX.X, op=OP.add)
                nc.vector.tensor_reduce(glimpse[:], gl_part[:].transpose([0, 2, 1]), axis=AX.X, op=OP.add)
                # g = glimpse @ W_out
                gT = work.tile([128, 2, 128], F32, tag="gT")
                for ci in range(2):
                    tp = psp1.tile([128, 128], F32, tag="tp")
                    nc.tensor.transpose(tp[:], glimpse[:, ci * 128:(ci + 1) * 128], ident[:])
                    nc.scalar.copy(gT[:, ci, :], tp[:])
                pg = psp.tile([128, D], F32, tag="ps")
                nc.tensor.matmul(pg[:], gT[:, 0, :], wo_sb[:, 0, :], start=True, stop=False)
                nc.tensor.matmul(pg[:], gT[:, 1, :], wo_sb[:, 1, :], start=False, stop=True)
                g_sb = work.tile([128, D], F32, tag="g_sb")
                nc.scalar.copy(g_sb[:], pg[:])
                gb = g_sb[:].rearrange("p (o f) -> p o f", o=1).broadcast_to([128, CHUNK, D])
                # --- logits: g . lK ---
                for c in range(NCH):
                    lc = stream.tile([128, CHUNK, D], F32, tag="stream")
                    nc.sync.dma_start(lc[:], kvl_d[:, c * CHUNK:(c + 1) * CHUNK, 2 * D:3 * D])
                    pr = prodp.tile([128, CHUNK, D], F32, tag="prod")
                    nc.gpsimd.tensor_tensor(pr[:], lc[:], gb, op=OP.mult)
                    nc.vector.tensor_reduce(logits[:, c * CHUNK:(c + 1) * CHUNK], pr[:], axis=AX.X, op=OP.add)
                # tanh clip, mask, log_softmax
                lgt = work.tile([128, N], F32, tag="lgt")
                nc.scalar.activation(lgt[:], logits[:], ACTF.Tanh, scale=ISD256)
                nc.vector.tensor_scalar(logits[:], lgt[:], 10.0, None, op0=OP.mult)
                nc.vector.tensor_tensor(logits[:], logits[:], amask[:], op=OP.add)
                m1 = work.tile([128, 1], F32, tag="m1")
                nc.vector.tensor_reduce(m1[:], logits[:], axis=AX.X, op=OP.max)
                shl = work.tile([128, N], F32, tag="shl")
                nc.vector.tensor_tensor(shl[:], logits[:], m1[:].broadcast_to([128, N]), op=OP.subtract)
                pexp = work.tile([128, N], F32, tag="pexp")
                s1 = work.tile([128, 1], F32, tag="s1")
                nc.scalar.activation(pexp[:], shl[:], ACTF.Exp, accum_out=s1[:])
                ls = work.tile([128, 1], F32, tag="ls")
                nc.scalar.activation(ls[:], s1[:], ACTF.Ln)
                lp = work.tile([128, N], F32, tag="lp")
                nc.vector.tensor_tensor(lp[:], shl[:], ls[:].broadcast_to([128, N]), op=OP.subtract)
                nc.sync.dma_start(out[:, bass.ds(s * N, N)], lp[:])
                # --- argmax + state update ---
                mx8 = work.tile([128, 8], F32, tag="mx8")
                nc.vector.max(mx8[:], logits[:])
                ix8 = work.tile([128, 8], dt.uint32, tag="ix8")
                nc.vector.max_index(ix8[:], mx8[:], logits[:])
                sel = work.tile([128, 1], dt.int32, tag="sel")
                nc.vector.tensor_copy(sel[:], ix8[:, 0:1])
                ohi = work.tile([128, N], dt.int32, tag="ohi")
                nc.vector.tensor_tensor(ohi[:], iota_n[:], sel[:].broadcast_to([128, N]), op=OP.is_equal)
                ohf = work.tile([128, N], F32, tag="ohf")
                nc.vector.tensor_copy(ohf[:], ohi[:])
                nc.vector.tensor_scalar(ohf[:], ohf[:], NEG, None, op0=OP.mult)
                nc.vector.tensor_tensor(amask[:], amask[:], ohf[:], op=OP.add)
                # gather next cur + q
                offs = work.tile([128, 1], dt.int32, tag="offs")
                nc.vector.tensor_tensor(offs[:], iota_row[:], sel[:], op=OP.add)
                nc.gpsimd.indirect_dma_start(
                    out=cur_sb[:], out_offset=None,
                    in_=emb2_d, in_offset=IndirectOffsetOnAxis(ap=offs[:], axis=0))
                q_from_cur()

            tc.For_i_unrolled(0, T, 1, step_body, max_unroll=4)

    nc.compile()
    return nc


_CACHE = {}


def kernel(**inputs) -> np.ndarray:
    if "nc" not in _CACHE:
        _CACHE["nc"] = _build()
    nc = _CACHE["nc"]

    emb = np.ascontiguousarray(np.asarray(inputs["embeddings"], np.float32))
    shared = {
        "pref_embed": np.asarray(inputs["pref_embed"], np.float32),
        "W_node": np.asarray(inputs["W_node"], np.float32),
        "W_fixed": np.asarray(inputs["W_fixed"], np.float32),
        "W_step": np.asarray(inputs["W_step"], np.float32),
        "W_out": np.asarray(inputs["W_out"], np.float32),
    }
    in_maps = []
    for i in range(NCORES):
        m = {"embeddings": emb[i * BS:(i + 1) * BS]}
        m.update(shared)
        in_maps.append(m)

    res = run_bass_kernel_spmd(nc, in_maps, list(range(NCORES)))
    outs = [res.results[i]["log_p"].reshape(BS, T, N) for i in range(NCORES)]
    return np.concatenate(outs, axis=0)


if __name__ == "__main__":
    z = np.load("inputs.npz")
    inp = {k: z[k] for k in z.files}
    o = kernel(**inp)
    print("kernel output", o.shape, o.dtype)
    np.save("kernel_out.npy", o)



# revision 15
# speedup vs baseline: 1.0039x; 1.0039x over previous
"""Trainium2 Bass kernel for nn_AttentionModel (greedy pointer-attention decode).

v2: visited-node compaction. Per decode step t only the ~200-t unvisited
nodes' K/V/lK rows are streamed from HBM via indirect (gather) DMA, driven by
an on-device compacted index list; step t+1's gathers are prefetched during
step t using a one-stale-entry list (the stale column is masked). q is
updated via a precomputed P = emb2 @ W_step[256:] row gather. The device
writes compacted log_p rows plus argmax positions and lse per step; the host
replays the (deterministic) index-list bookkeeping and scatters rows to the
full (B, 199, 200) output.

Per-core dataflow (batch-on-partitions, b=128):
  precompute: emb2 = emb + pref; kvl = emb2 @ W_node -> DRAM; P = emb2 @ W2
              -> DRAM; fixed2 = mean(emb2) @ W_fixed + first @ W_step[:256]
  per step  : 3 column-gathers (K|V|lK) of compacted rows; DVE does products
              + segmented reduces; ACT exp/tanh/ln; PE glimpse @ W_out.
"""
import numpy as np

import concourse.bass as bass
from concourse import bacc
import concourse.tile as tile
from concourse import mybir
from concourse.bass import IndirectOffsetOnAxis
from concourse.bass_utils import run_bass_kernel_spmd

dt = mybir.dt
F32 = dt.float32
I32 = dt.int32
AX = mybir.AxisListType
OP = mybir.AluOpType
ACTF = mybir.ActivationFunctionType

B, N, D, H = 1024, 200, 256, 8
d = D // H                      # 32
NCORES = 8
BS = B // NCORES                # 128 items per core
T = N - 1                       # 199 decode steps
START = 24
NEG = -1e9
CH = 16                         # rows per gather chunk
ISD32 = float(np.float32(1.0 / np.sqrt(32.0)))
ISD256 = 0.0625


def width(t):
    # gathered width of step t (one stale column for t >= 1)
    return 199 if t == 0 else 200 - t


def _build():
    nc = bacc.Bacc("TRN2", target_bir_lowering=False, debug=False)

    emb_in = nc.dram_tensor("embeddings", [BS, N, D], F32, kind="ExternalInput").ap()
    pref_in = nc.dram_tensor("pref_embed", [D], F32, kind="ExternalInput").ap()
    wnode_in = nc.dram_tensor("W_node", [D, 3 * D], F32, kind="ExternalInput").ap()
    wfix_in = nc.dram_tensor("W_fixed", [D, D], F32, kind="ExternalInput").ap()
    wstep_in = nc.dram_tensor("W_step", [2 * D, D], F32, kind="ExternalInput").ap()
    wout_in = nc.dram_tensor("W_out", [D, D], F32, kind="ExternalInput").ap()

    lp_out = nc.dram_tensor("lp_c", [BS, T * N], F32, kind="ExternalOutput").ap()
    idx_out = nc.dram_tensor("idx_c", [BS, T], I32, kind="ExternalOutput").ap()
    lse_out = nc.dram_tensor("lse_c", [BS, T], F32, kind="ExternalOutput").ap()

    emb2_d = nc.dram_tensor("emb2_d", [BS * N, D], F32).ap()
    kvl_d = nc.dram_tensor("kvl_d", [BS * N, 3 * D], F32).ap()
    p_d = nc.dram_tensor("p_d", [BS * N, D], F32).ap()
    of16_d0 = nc.dram_tensor("of16_d0", [128, 201], dt.int16).ap()
    of16_d1 = nc.dram_tensor("of16_d1", [128, 201], dt.int16).ap()
    of16_d = [of16_d0, of16_d1]

    with tile.TileContext(nc) as tc:
        with tc.tile_pool(name="wpool", bufs=1) as wpool, \
             tc.tile_pool(name="psum", bufs=2, space="PSUM") as psp, \
             tc.tile_pool(name="psum1", bufs=2, space="PSUM") as psp1:

            # ------------- persistent decode-state tiles -------------
            wo_sb = wpool.tile([128, 2, D], F32)        # W_out c-tiles
            nc.sync.dma_start(wo_sb[:, 0, :], wout_in[0:128, :])
            nc.sync.dma_start(wo_sb[:, 1, :], wout_in[128:256, :])

            ident = wpool.tile([128, 128], F32)         # identity for PE transpose
            io_c = wpool.tile([128, 128], I32)
            nc.gpsimd.iota(io_c[:], pattern=[[1, 128]], channel_multiplier=0)
            io_r = wpool.tile([128, 1], I32)
            nc.gpsimd.iota(io_r[:], pattern=[[0, 1]], channel_multiplier=1)
            id_i = wpool.tile([128, 128], I32)
            nc.vector.tensor_tensor(id_i[:], io_c[:], io_r[:].broadcast_to([128, 128]), op=OP.is_equal)
            nc.vector.tensor_copy(ident[:], id_i[:])

            iota201 = wpool.tile([128, 201], I32)       # 0..200 per partition
            nc.gpsimd.iota(iota201[:], pattern=[[1, 201]], channel_multiplier=0)
            rowbase = wpool.tile([128, 1], I32)         # p*N
            nc.gpsimd.iota(rowbase[:], pattern=[[0, 1]], channel_multiplier=N)

            # compacted node list double buffers + row-offset buffers
            clist0 = wpool.tile([128, 201], I32)
            clist1 = wpool.tile([128, 201], I32)
            clist = [clist0, clist1]
            # wrapped int16 global index tiles for dma_gather (double-buffered)
            idxw0 = wpool.tile([128, 8 * 201], dt.int16)
            idxw1 = wpool.tile([128, 8 * 201], dt.int16)
            idxw = [idxw0, idxw1]
            # clist[0][j] = j + (j >= START), widths beyond current W unused
            ge24 = wpool.tile([128, 201], I32)
            nc.vector.tensor_scalar(ge24[:], iota201[:], START, None, op0=OP.is_ge)
            nc.vector.tensor_tensor(clist[0][:], iota201[:], ge24[:], op=OP.add)

            fixed2 = wpool.tile([128, D], F32)
            q_sb = wpool.tile([128, D], F32)
            compat = wpool.tile([128, N, H], F32)
            attn = wpool.tile([128, N, H], F32)
            glp = wpool.tile([128, (199 + CH - 1) // CH, D], F32)
            glimpse = wpool.tile([128, D], F32)
            g_sb = wpool.tile([128, D], F32)
            logits = wpool.tile([128, N], F32)
            maskf = wpool.tile([128, N], F32)
            m_pos = wpool.tile([128, 1], I32)
            sel_i = wpool.tile([128, 1], I32)
            idx_acc = wpool.tile([128, T], I32)
            lse_acc = wpool.tile([128, T], F32)
            pq_sb = wpool.tile([128, D], F32)

            # ---------------- precompute ----------------
            with tc.tile_pool(name="prepool", bufs=1) as prep, \
                 tc.tile_pool(name="prework", bufs=2) as work:
                wn_sb = prep.tile([128, 2, 3 * D], F32)
                nc.sync.dma_start(wn_sb[:, 0, :], wnode_in[0:128, :])
                nc.sync.dma_start(wn_sb[:, 1, :], wnode_in[128:256, :])
                w2_sb = prep.tile([128, 2, D], F32)     # W_step[256:512]
                nc.sync.dma_start(w2_sb[:, 0, :], wstep_in[256:384, :])
                nc.sync.dma_start(w2_sb[:, 1, :], wstep_in[384:512, :])
                wf_sb = prep.tile([128, 2, D], F32)
                nc.sync.dma_start(wf_sb[:, 0, :], wfix_in[0:128, :])
                nc.sync.dma_start(wf_sb[:, 1, :], wfix_in[128:256, :])
                ws1_sb = prep.tile([128, 2, D], F32)    # W_step[0:256]
                nc.sync.dma_start(ws1_sb[:, 0, :], wstep_in[0:128, :])
                nc.sync.dma_start(ws1_sb[:, 1, :], wstep_in[128:256, :])
                pref_sb = prep.tile([128, D], F32)
                nc.sync.dma_start(
                    pref_sb[:],
                    pref_in.rearrange("(o f) -> o f", o=1).broadcast_to([128, D]))

                emb_rows = emb_in.rearrange("b n c -> (b n) c")   # [25600, 256]
                ROWT = BS * N // 128                              # 200 row-tiles

                def pre_body(rt):
                    erow = work.tile([128, D], F32, tag="erow")
                    nc.sync.dma_start(erow[:], emb_rows[bass.ds(rt * 128, 128), :])
                    e2 = work.tile([128, D], F32, tag="e2")
                    nc.vector.tensor_tensor(e2[:], erow[:], pref_sb[:], op=OP.add)
                    nc.sync.dma_start(emb2_d[bass.ds(rt * 128, 128), :], e2[:])
                    e2T = work.tile([128, 2, 128], F32, tag="e2T")
                    for ci in range(2):
                        tp = psp1.tile([128, 128], F32, tag="tp")
                        nc.tensor.transpose(tp[:], e2[:, ci * 128:(ci + 1) * 128], ident[:])
                        nc.vector.tensor_copy(e2T[:, ci, :], tp[:])
                    kv = work.tile([128, 3 * D], F32, tag="kv")
                    for fh in range(2):
                        pm = psp.tile([128, 384], F32, tag="ps")
                        nc.tensor.matmul(pm[:], e2T[:, 0, :], wn_sb[:, 0, fh * 384:(fh + 1) * 384], start=True, stop=False)
                        nc.tensor.matmul(pm[:], e2T[:, 1, :], wn_sb[:, 1, fh * 384:(fh + 1) * 384], start=False, stop=True)
                        nc.vector.tensor_copy(kv[:, fh * 384:(fh + 1) * 384], pm[:])
                    nc.sync.dma_start(kvl_d[bass.ds(rt * 128, 128), :], kv[:])
                    pm2 = psp.tile([128, 384], F32, tag="ps")
                    nc.tensor.matmul(pm2[:, 0:D], e2T[:, 0, :], w2_sb[:, 0, :], start=True, stop=False)
                    nc.tensor.matmul(pm2[:, 0:D], e2T[:, 1, :], w2_sb[:, 1, :], start=False, stop=True)
                    prow = work.tile([128, D], F32, tag="prow")
                    nc.scalar.copy(prow[:], pm2[:, 0:D])
                    nc.sync.dma_start(p_d[bass.ds(rt * 128, 128), :], prow[:])

                tc.For_i_unrolled(0, ROWT, 1, pre_body, max_unroll=4)

                # fixed2 = mean(emb2) @ W_fixed + first @ W_step[:256]
                macc = prep.tile([128, D], F32)
                first_sb = prep.tile([128, D], F32)
                emb2_bnc = emb2_d.rearrange("(b n) c -> b n c", b=BS)
                PRECH = 20
                for c in range(N // PRECH):
                    ech = work.tile([128, PRECH, D], F32, tag="ech")
                    nc.sync.dma_start(ech[:], emb2_bnc[:, c * PRECH:(c + 1) * PRECH, :])
                    part = work.tile([128, D], F32, tag="mpart")
                    nc.vector.tensor_reduce(part[:], ech[:].transpose([0, 2, 1]), axis=AX.X, op=OP.add)
                    if c == 0:
                        nc.vector.tensor_copy(macc[:], part[:])
                    else:
                        nc.vector.tensor_tensor(macc[:], macc[:], part[:], op=OP.add)
                nc.vector.tensor_scalar(macc[:], macc[:], 1.0 / N, None, op0=OP.mult)
                nc.sync.dma_start(first_sb[:], emb2_bnc[:, START, :])

                fT = work.tile([128, 2, 128], F32, tag="fT")
                mT = work.tile([128, 2, 128], F32, tag="mT")
                for ci in range(2):
                    tp = psp1.tile([128, 128], F32, tag="tp")
                    nc.tensor.transpose(tp[:], macc[:, ci * 128:(ci + 1) * 128], ident[:])
                    nc.vector.tensor_copy(mT[:, ci, :], tp[:])
                    tp2 = psp1.tile([128, 128], F32, tag="tp")
                    nc.tensor.transpose(tp2[:], first_sb[:, ci * 128:(ci + 1) * 128], ident[:])
                    nc.vector.tensor_copy(fT[:, ci, :], tp2[:])
                pf = psp.tile([128, 384], F32, tag="ps")
                nc.tensor.matmul(pf[:, 0:D], mT[:, 0, :], wf_sb[:, 0, :], start=True, stop=False)
                nc.tensor.matmul(pf[:, 0:D], mT[:, 1, :], wf_sb[:, 1, :], start=False, stop=False)
                nc.tensor.matmul(pf[:, 0:D], fT[:, 0, :], ws1_sb[:, 0, :], start=False, stop=False)
                nc.tensor.matmul(pf[:, 0:D], fT[:, 1, :], ws1_sb[:, 1, :], start=False, stop=True)
                nc.vector.tensor_copy(fixed2[:], pf[:, 0:D])

                # q(t=0) = fixed2 + P[:, START]
                nc.sync.dma_start(pq_sb[:], p_d.rearrange("(b n) c -> b n c", b=BS)[:, START, :])
                nc.vector.tensor_tensor(q_sb[:], pq_sb[:], fixed2[:], op=OP.add)

            # ---------------- decode ----------------
            with tc.tile_pool(name="kst", bufs=2) as kpool, \
                 tc.tile_pool(name="vst", bufs=2) as vpool, \
                 tc.tile_pool(name="lst", bufs=2) as lpool, \
                 tc.tile_pool(name="prod", bufs=2) as prodp, \
                 tc.tile_pool(name="dwork", bufs=2) as work:

                def chunks(t):
                    w = width(t)
                    return [(c * CH, min((c + 1) * CH, w) - c * CH)
                            for c in range((w + CH - 1) // CH)]

                def build_idxw(buf_i, src_list, wv):
                    """idxw[buf_i][q, 8j+s] = src_list[16s+q%16, j] + (16s+q%16)*N
                    as int16, wrapped for dma_gather and replicated per 16-group."""
                    o32 = work.tile([128, 201], I32, tag="o32")
                    nc.vector.tensor_tensor(
                        o32[:, 0:wv], src_list[:, 0:wv],
                        rowbase[:].broadcast_to([128, wv]), op=OP.add)
                    o16 = work.tile([128, 201], dt.int16, tag="o16")
                    nc.vector.tensor_copy(o16[:, 0:wv], o32[:, 0:wv])
                    nc.sync.dma_start(of16_d[buf_i][:, 0:wv], o16[:, 0:wv])
                    i8 = work.tile([128, 8, 201], dt.int16, tag="i8")
                    for s_ in range(8):
                        nc.sync.dma_start(
                            i8[:, s_, 0:wv],
                            of16_d[buf_i][s_ * 16:(s_ + 1) * 16, 0:wv]
                            .rearrange("(o r) j -> o r j", o=1).broadcast_to([8, 16, wv]))
                    nc.vector.tensor_copy(
                        idxw[buf_i][:, 0:8 * wv].rearrange("p (j s) -> p j s", s=8),
                        i8[:, :, 0:wv].transpose([0, 2, 1]))

                pend = {}   # (t, stream, chunk) -> tile

                def emit_gather(t, stream, c0, cw, pool, eo):
                    tl = pool.tile([128, CH, D], F32, tag="st")
                    ib = idxw[(t - 1) % 2] if t >= 1 else idxw[0]
                    nc.gpsimd.dma_gather(
                        tl[:, 0:cw, :], kvl_d[:, eo:eo + D],
                        ib[:, 8 * c0:8 * (c0 + cw)],
                        128 * cw, 128 * cw, D, elem_step=3 * D,
                        single_packet=False)
                    pend[(t, stream, c0)] = tl

                build_idxw(0, clist[0][:], 199)

                # initial gathers for step 0 (all three streams)
                for (c0, cw) in chunks(0):
                    emit_gather(0, 0, c0, cw, kpool, 0)
                for (c0, cw) in chunks(0):
                    emit_gather(0, 1, c0, cw, vpool, D)
                for (c0, cw) in chunks(0):
                    emit_gather(0, 2, c0, cw, lpool, 2 * D)

                for t in range(T):
                    w = width(t)
                    chs = chunks(t)
                    chs_n = chunks(t + 1) if t + 1 < T else []
                    nch = len(chs)
                    qb = q_sb[:].rearrange("p (o f) -> p o f", o=1)

                    # --- compat phase (K) ---
                    for ci, (c0, cw) in enumerate(chs):
                        kb = pend.pop((t, 0, c0))
                        pr = prodp.tile([128, CH, D], F32, tag="pr")
                        nc.vector.tensor_tensor(
                            pr[:, 0:cw, :], kb[:, 0:cw, :],
                            qb.broadcast_to([128, cw, D]), op=OP.mult)
                        nc.vector.tensor_reduce(
                            compat[:, c0:c0 + cw, :],
                            pr[:, 0:cw, :].rearrange("p n (h e) -> p n h e", h=H),
                            axis=AX.X, op=OP.add)
                        if ci < len(chs_n):
                            emit_gather(t + 1, 0, chs_n[ci][0], chs_n[ci][1], kpool, 0)

                    # stale-column mask (NEG at m_pos), then scale+mask compat
                    if t >= 1:
                        ohi = work.tile([128, N], I32, tag="ohi")
                        nc.vector.tensor_tensor(
                            ohi[:, 0:w], iota201[:, 0:w],
                            m_pos[:].broadcast_to([128, w]), op=OP.is_equal)
                        nc.vector.tensor_scalar(maskf[:, 0:w], ohi[:, 0:w], NEG, None, op0=OP.mult)
                        nc.vector.scalar_tensor_tensor(
                            compat[:, 0:w, :], compat[:, 0:w, :], ISD32,
                            maskf[:, 0:w].rearrange("p (w o) -> p w o", o=1).broadcast_to([128, w, H]),
                            op0=OP.mult, op1=OP.add)
                    else:
                        nc.vector.tensor_scalar(compat[:, 0:w, :], compat[:, 0:w, :], ISD32, None, op0=OP.mult)

                    # softmax over gathered axis, per head
                    mh = work.tile([128, H], F32, tag="mh")
                    nc.vector.tensor_reduce(mh[:], compat[:, 0:w, :].transpose([0, 2, 1]), axis=AX.X, op=OP.max)
                    nc.vector.tensor_tensor(
                        compat[:, 0:w, :], compat[:, 0:w, :],
                        mh[:].rearrange("p (o h) -> p o h", o=1).broadcast_to([128, w, H]),
                        op=OP.subtract)
                    nc.scalar.activation(attn[:, 0:w, :], compat[:, 0:w, :], ACTF.Exp)
                    sh = work.tile([128, H], F32, tag="sh")
                    nc.vector.tensor_reduce(sh[:], attn[:, 0:w, :].transpose([0, 2, 1]), axis=AX.X, op=OP.add)
                    rh = work.tile([128, H], F32, tag="rh")
                    nc.vector.reciprocal(rh[:], sh[:])
                    nc.vector.scalar_tensor_tensor(
                        attn[:, 0:w, :], attn[:, 0:w, :], 1.0,
                        rh[:].rearrange("p (o h) -> p o h", o=1).broadcast_to([128, w, H]),
                        op0=OP.mult, op1=OP.mult)

                    # --- glimpse phase (V) ---
                    for ci, (c0, cw) in enumerate(chs):
                        vb = pend.pop((t, 1, c0))
                        pr = prodp.tile([128, CH, D], F32, tag="pr")
                        nc.vector.tensor_tensor(
                            pr[:, 0:cw, :].rearrange("p n (h e) -> p n h e", h=H),
                            vb[:, 0:cw, :].rearrange("p n (h e) -> p n h e", h=H),
                            attn[:, c0:c0 + cw, :].rearrange("p n (h o) -> p n h o", o=1).broadcast_to([128, cw, H, d]),
                            op=OP.mult)
                        nc.vector.tensor_reduce(
                            glp[:, ci, :], pr[:, 0:cw, :].transpose([0, 2, 1]),
                            axis=AX.X, op=OP.add)
                        if ci < len(chs_n):
                            emit_gather(t + 1, 1, chs_n[ci][0], chs_n[ci][1], vpool, D)
                    nc.vector.tensor_reduce(
                        glimpse[:], glp[:, 0:nch, :].transpose([0, 2, 1]), axis=AX.X, op=OP.add)

                    # g = glimpse @ W_out
                    gT = work.tile([128, 2, 128], F32, tag="gT")
                    for ci2 in range(2):
                        tp = psp1.tile([128, 128], F32, tag="tp")
                        nc.tensor.transpose(tp[:], glimpse[:, ci2 * 128:(ci2 + 1) * 128], ident[:])
                        nc.scalar.copy(gT[:, ci2, :], tp[:])
                    pg = psp.tile([128, 384], F32, tag="ps")
                    nc.tensor.matmul(pg[:, 0:D], gT[:, 0, :], wo_sb[:, 0, :], start=True, stop=False)
                    nc.tensor.matmul(pg[:, 0:D], gT[:, 1, :], wo_sb[:, 1, :], start=False, stop=True)
                    nc.scalar.copy(g_sb[:], pg[:, 0:D])
                    gb = g_sb[:].rearrange("p (o f) -> p o f", o=1)

                    # --- logits phase (lK) ---
                    for ci, (c0, cw) in enumerate(chs):
                        lb = pend.pop((t, 2, c0))
                        pr = prodp.tile([128, CH, D], F32, tag="pr")
                        nc.vector.tensor_tensor(
                            pr[:, 0:cw, :], lb[:, 0:cw, :],
                            gb.broadcast_to([128, cw, D]), op=OP.mult)
                        nc.vector.tensor_reduce(
                            logits[:, c0:c0 + cw], pr[:, 0:cw, :], axis=AX.X, op=OP.add)
                        if ci < len(chs_n):
                            emit_gather(t + 1, 2, chs_n[ci][0], chs_n[ci][1], lpool, 2 * D)

                    # tanh clip, stale mask, log_softmax
                    lgt = work.tile([128, N], F32, tag="lgt")
                    nc.scalar.activation(lgt[:, 0:w], logits[:, 0:w], ACTF.Tanh, scale=ISD256)
                    if t >= 1:
                        nc.vector.scalar_tensor_tensor(
                            logits[:, 0:w], lgt[:, 0:w], 10.0, maskf[:, 0:w],
                            op0=OP.mult, op1=OP.add)
                    else:
                        nc.vector.tensor_scalar(logits[:, 0:w], lgt[:, 0:w], 10.0, None, op0=OP.mult)
                    m1 = work.tile([128, 1], F32, tag="m1")
                    nc.vector.tensor_reduce(m1[:], logits[:, 0:w], axis=AX.X, op=OP.max)
                    shl = work.tile([128, N], F32, tag="shl")
                    nc.vector.tensor_tensor(shl[:, 0:w], logits[:, 0:w], m1[:].broadcast_to([128, w]), op=OP.subtract)
                    pexp = work.tile([128, N], F32, tag="pexp")
                    s1 = work.tile([128, 1], F32, tag="s1")
                    nc.scalar.activation(pexp[:, 0:w], shl[:, 0:w], ACTF.Exp, accum_out=s1[:])
                    ls = work.tile([128, 1], F32, tag="ls")
                    nc.scalar.activation(ls[:], s1[:], ACTF.Ln)
                    lp = work.tile([128, N], F32, tag="lp")
                    nc.vector.tensor_tensor(lp[:, 0:w], shl[:, 0:w], ls[:].broadcast_to([128, w]), op=OP.subtract)
                    nc.sync.dma_start(lp_out[:, bass.ds(t * N, w)], lp[:, 0:w])
                    nc.vector.tensor_tensor(lse_acc[:, t:t + 1], m1[:], ls[:], op=OP.add)

                    # argmax over masked logits (pad to >=8 wide for vector.max)
                    wa = max(w, 8)
                    if w < 8:
                        nc.vector.memset(logits[:, w:8], NEG)
                    mx8 = work.tile([128, 8], F32, tag="mx8")
                    nc.vector.max(mx8[:], logits[:, 0:wa])
                    ix8 = work.tile([128, 8], dt.uint32, tag="ix8")
                    nc.vector.max_index(ix8[:], mx8[:], logits[:, 0:wa])
                    idx = work.tile([128, 1], I32, tag="idx")
                    nc.vector.tensor_copy(idx[:], ix8[:, 0:1])
                    nc.vector.tensor_copy(idx_acc[:, t:t + 1], idx[:])

                    if t + 1 >= T:
                        break

                    # --- bookkeeping for next step ---
                    # m_pos(next) = idx - (idx > m_pos)   [t=0: m_pos = idx]
                    if t == 0:
                        nc.vector.tensor_copy(m_pos[:], idx[:])
                    else:
                        cmp = work.tile([128, 1], I32, tag="cmp")
                        nc.vector.tensor_tensor(cmp[:], idx[:], m_pos[:], op=OP.is_gt)
                        nc.vector.tensor_tensor(m_pos[:], idx[:], cmp[:], op=OP.subtract)

                    # sel = glist[idx]; glist for step t is clist[(t-1)%2] (t<=1 -> clist[0])
                    gl = clist[0] if t <= 1 else clist[(t - 1) % 2]
                    ohs = work.tile([128, N], I32, tag="ohs")
                    nc.vector.tensor_tensor(
                        ohs[:, 0:w], iota201[:, 0:w], idx[:].broadcast_to([128, w]), op=OP.is_equal)
                    prs = work.tile([128, N], I32, tag="prs")
                    nc.vector.tensor_tensor(prs[:, 0:w], ohs[:, 0:w], gl[:, 0:w], op=OP.mult)
                    with nc.allow_low_precision(reason="exact one-hot int32 dot"):
                        nc.vector.tensor_reduce(sel_i[:], prs[:, 0:w], axis=AX.X, op=OP.add)

                    # remove element m_pos from clist[t%2] -> clist[(t+1)%2]
                    wn = width(t + 2) if t + 2 < T + 1 else width(t + 1) - 1
                    src = clist[0] if t == 0 else clist[t % 2]
                    dstl = clist[(t + 1) % 2]
                    geb = work.tile([128, N], I32, tag="geb")
                    nc.vector.tensor_tensor(
                        geb[:, 0:wn], iota201[:, 0:wn], m_pos[:].broadcast_to([128, wn]), op=OP.is_ge)
                    nc.vector.select(dstl[:, 0:wn], geb[:, 0:wn], src[:, 1:wn + 1], src[:, 0:wn])
                    build_idxw((t + 1) % 2, dstl[:], wn)

                    # q(next) = fixed2 + P[sel]
                    offp = work.tile([128, 1], I32, tag="offp")
                    nc.vector.tensor_tensor(offp[:], sel_i[:], rowbase[:], op=OP.add)
                    nc.gpsimd.indirect_dma_start(
                        out=pq_sb[:], out_offset=None,
                        in_=p_d, in_offset=IndirectOffsetOnAxis(ap=offp[:], axis=0))
                    nc.vector.tensor_tensor(q_sb[:], pq_sb[:], fixed2[:], op=OP.add)

                nc.sync.dma_start(idx_out[:, :], idx_acc[:])
                nc.sync.dma_start(lse_out[:, :], lse_acc[:])

    nc.compile()
    return nc


_CACHE = {}


def _host_expand(lp_c, idx_c, lse_c):
    """Replay the device index bookkeeping; scatter compacted rows to full."""
    Bf = lp_c.shape[0]
    out = np.empty((Bf, T, N), np.float32)
    ar = np.arange(Bf)
    clist = [np.zeros((Bf, 201), np.int64) for _ in range(2)]
    j = np.arange(201)
    clist[0][:] = j + (j >= START)
    m_pos = np.zeros(Bf, np.int64)
    for t in range(T):
        w = width(t)
        gl = clist[0] if t <= 1 else clist[(t - 1) % 2]
        cols = gl[:, :w]
        fill = (-1e9 - lse_c[:, t]).astype(np.float32)
        out[:, t, :] = fill[:, None]
        np.put_along_axis(out[:, t, :], cols, lp_c[:, t, :w], axis=1)
        if t >= 1:
            # stale column: force exact fill value (device wrote approx -1e9)
            stale_col = cols[ar, m_pos]
            out[ar, t, stale_col] = fill
        idx = idx_c[:, t].astype(np.int64)
        if t + 1 >= T:
            break
        m_pos = idx if t == 0 else idx - (idx > m_pos)
        wn = width(t + 2) if t + 2 < T + 1 else width(t + 1) - 1
        src = clist[0] if t == 0 else clist[t % 2]
        dstl = clist[(t + 1) % 2]
        jn = np.arange(wn)
        take = jn[None, :] + (jn[None, :] >= m_pos[:, None])
        dstl[:, :wn] = np.take_along_axis(src, take, axis=1)
    return out.reshape(Bf, T, N)


def kernel(**inputs) -> np.ndarray:
    if "nc" not in _CACHE:
        _CACHE["nc"] = _build()
    nc = _CACHE["nc"]

    emb = np.ascontiguousarray(np.asarray(inputs["embeddings"], np.float32))
    shared = {
        "pref_embed": np.asarray(inputs["pref_embed"], np.float32),
        "W_node": np.asarray(inputs["W_node"], np.float32),
        "W_fixed": np.asarray(inputs["W_fixed"], np.float32),
        "W_step": np.asarray(inputs["W_step"], np.float32),
        "W_out": np.asarray(inputs["W_out"], np.float32),
    }
    in_maps = []
    for i in range(NCORES):
        m = {"embeddings": emb[i * BS:(i + 1) * BS]}
        m.update(shared)
        in_maps.append(m)

    res = run_bass_kernel_spmd(nc, in_maps, list(range(NCORES)))
    outs = []
    for i in range(NCORES):
        r = res.results[i]
        outs.append(_host_expand(
            r["lp_c"].reshape(BS, T, N), r["idx_c"], r["lse_c"]))
    return np.concatenate(outs, axis=0)


if __name__ == "__main__":
    z = np.load("inputs.npz")
    inp = {k: z[k] for k in z.files}
    o = kernel(**inp)
    print("kernel output", o.shape, o.dtype)
    np.save("kernel_out.npy", o)
